# revision 1
# baseline (speedup 1.0000x reference)
"""Deformable-conv module (offset conv -> bilinear deform conv -> sync-BN -> ReLU)
as a Trainium2 Bass kernel on 8 NeuronCores.

Sharding: core = (batch b, pixel-half ph).  Each core computes the full
256-channel output for 2048 pixels (32 image rows) of one batch image.
Full C=256 contraction is local, so no partial-sum exchange is needed;
only BN statistics cross cores (one 4KB AllReduce).

Bilinear sampling: x is host-padded into an 80x80 zero-extended grid and
packed as bf16 (value, right-neighbor) pairs in fp32 containers.  One
ap_gather index fetches both x-corners of a row; the row+1 corners come
from the same index into the grid shifted by one row.  Zero padding makes
all out-of-image corners contribute exactly 0, so no validity masks are
needed.  The 4-corner bilinear sum is folded into the deform GEMM's
contraction (each corner stream is a moving operand; PSUM accumulates).
"""
import sys, os, time

sys.path.insert(0, "/opt/trn_rl_repo")

import numpy as np
import ml_dtypes

import concourse.bacc as bacc
import concourse.tile as tile
from concourse import mybir
from concourse import library_config
from concourse.alu_op_type import AluOpType
from concourse.bass_utils import run_bass_kernel_spmd

F32 = mybir.dt.float32
BF16 = mybir.dt.bfloat16
I16 = mybir.dt.int16
AF = mybir.ActivationFunctionType
AX = mybir.AxisListType
MM = mybir.MatmulPerfMode

B, C, H, W, O = 4, 256, 64, 64, 256
K = 3
K2 = 9
EPS = 1e-5
PADE = 8          # extension pad on each side of the image
EG = H + 2 * PADE  # 80: extended grid edge
EGF = EG * EG      # 6400 ext pixels
PREB = 648         # leading zero rows so biased indices need no -648
XTROWS = PREB + EGF  # 7048 rows in the HBM gather table
NELEM = 6967       # gather source rows (covers max biased index 6966)
MAGIC = 12582912.0  # 1.5 * 2**23: fp32 round-to-int trick
HALF = 2048        # pixels per core
HROWS = HALF // W  # 32 image rows per core
NCORES = 8
NTOT = B * H * W   # BN normalization count


def build_program(num_devices=NCORES):
    nc = bacc.Bacc("TRN2", target_bir_lowering=False, debug=False,
                   num_devices=num_devices, num_swdge_queues=4)

    xp_d = nc.dram_tensor("xp", [128, XTROWS + EG, 2], F32,
                          kind="ExternalInput").ap()
    woff_d = nc.dram_tensor("woff", [2, 128, K2, 18], BF16, kind="ExternalInput").ap()
    wdef_d = nc.dram_tensor("wdef", [2, 128, K2, O], BF16, kind="ExternalInput").ap()
    kb_d = nc.dram_tensor("kb", [K2, 2, HALF], F32, kind="ExternalInput").ap()
    gb_d = nc.dram_tensor("gb", [2, 128, 2], F32, kind="ExternalInput").ap()
    out_d = nc.dram_tensor("out", [O, HALF], F32, kind="ExternalOutput").ap()
    # internal DRAM scratch for layout bounces
    widx_d = nc.dram_tensor("widx_s", [K2, HALF], I16).ap()
    wmap_d = nc.dram_tensor("wmap_s", [K2, 2, HALF], F32).ap()
    stats_in_d = nc.dram_tensor("stats_in", [128, 4], F32).ap()
    stats_out_d = nc.dram_tensor("stats_out", [128, 4], F32, addr_space="Shared").ap()

    with tile.TileContext(nc) as tc:
        with tc.tile_pool(name="per", bufs=1) as per, \
             tc.tile_pool(name="wb", bufs=2) as wbp, \
             tc.tile_pool(name="gt", bufs=2) as gtp, \
             tc.tile_pool(name="tt", bufs=2) as ttp, \
             tc.tile_pool(name="ps", bufs=2, space="PSUM") as psp:

            # ---- inputs to SBUF ----
            # d=2 gather table: element i = (cg0 container, cg1 container),
            # each container = bf16 (xe[i], xe[i+1]) pair in an fp32 word.
            # One gather request then fetches both channel groups, halving the
            # Q7 read-request count that rate-limits ap_gather.
            nc.gpsimd.load_library(library_config.ap_gather)
            XP2 = per.tile([128, XTROWS + EG, 2], F32, tag="xp2")
            nc.sync.dma_start(XP2[:], xp_d[:])
            WOFF = per.tile([128, 2, K2, 18], BF16, tag="woff")
            nc.sync.dma_start(WOFF[:, 0], woff_d[0])
            nc.sync.dma_start(WOFF[:, 1], woff_d[1])
            WDEF = per.tile([128, 2, K2, O], BF16, tag="wdef")
            nc.sync.dma_start(WDEF[:, 0], wdef_d[0])
            nc.sync.dma_start(WDEF[:, 1], wdef_d[1])
            GBt = per.tile([128, 2, 2], F32, tag="gb")
            nc.sync.dma_start(GBt[:, 0], gb_d[0])
            nc.sync.dma_start(GBt[:, 1], gb_d[1])

            # [part, 80 ext rows, 80 cols] view for conv windows: stride-4
            # bf16 view into the packed table (bf16 4i+2cg = xe_cg[i])
            xvb = XP2[:, PREB:PREB + EGF].bitcast(BF16)\
                .rearrange("p (r c) e -> p r (c e)", c=EG)
            xv = [xvb[:, :, 2 * cg::4] for cg in range(2)]
            xstep = 1

            # ---- offset conv: two GEMMs (y comps, x comps) ----
            # psum partitions = 9 taps; moving = 32x64 image window, 512-col chunks
            ps_y = psp.tile([K2, HALF], F32, tag="ps")
            ps_x = psp.tile([K2, HALF], F32, tag="ps")
            n_mm = 0
            for comp, pst in ((0, ps_y), (1, ps_x)):
                for cg in range(2):
                    for kk in range(K2):
                        dy, dx = kk // K - 1, kk % K - 1
                        r0 = PADE + dy   # ext row of first window row (ph folded in kb? no: rows differ per core!)
                        # NOTE: per-core row offset handled via kb; but the
                        # window itself must read this core's rows.  The row
                        # base depends on ph which is NOT known at build time,
                        # so we bake it via a per-core DRAM input instead?  No:
                        # all cores run the same program; we make the window
                        # row base a *program constant* = PADE+dy and add the
                        # per-core 32-row offset by shifting the data on host?
                        # Host shifts: xp is the full 80x80 grid; instead the
                        # moving AP below uses rows [PADE+dy+ROWOFF ...] with
                        # ROWOFF supplied via host-rolled kb... -> resolved by
                        # building xp PER CORE with the core's 32-row window
                        # centered: see host prep (xe rolled so that rows
                        # [PADE..PADE+32) are this core's rows).
                        stat = WOFF[:, cg, kk, 9 * comp:9 * comp + 9]
                        for q in range(4):
                            rq = r0 + 8 * q
                            mov = xv[cg][:, rq:rq + 8,
                                         xstep * (PADE + dx):
                                         xstep * (PADE + dx) + xstep * W:xstep]
                            nc.tensor.matmul(
                                pst[:, 512 * q:512 * (q + 1)], stat, mov,
                                start=(cg == 0 and kk == 0),
                                stop=(cg == 1 and kk == K2 - 1))
                            n_mm += 1

            # ---- coordinate math ----
            # A = py + 15.5 (kb holds tap offset + base + 15.5); add conv psums
            AYX = per.tile([K2, 2, HALF], F32, tag="ayx")
            nc.sync.dma_start(AYX[:], kb_d[:])
            nc.vector.tensor_tensor(AYX[:, 0], AYX[:, 0], ps_y[:], AluOpType.add)
            nc.vector.tensor_tensor(AYX[:, 1], AYX[:, 1], ps_x[:], AluOpType.add)
            # FI = round(A) = floor(py) + 16 (fp32 magic-number round)
            FI = per.tile([K2, 2, HALF], F32, tag="fi")
            nc.vector.tensor_scalar(FI[:], AYX[:], MAGIC, -MAGIC,
                                    AluOpType.add, AluOpType.add)
            # D = A - FI in (-0.5, 0.5]; frac = D + 0.5, 1-frac = 0.5 - D
            nc.vector.tensor_tensor(AYX[:], AYX[:], FI[:], AluOpType.subtract)
            # clip FI to [8, 86] (= coord in [-8, 70]), in place
            nc.vector.tensor_scalar(FI[:], FI[:], 8.0, 86.0,
                                    AluOpType.max, AluOpType.min)
            # gather index = 80*ycl + xcl (maps into the PREB-padded grid)
            nc.vector.scalar_tensor_tensor(FI[:, 1], FI[:, 0], 80.0, FI[:, 1],
                                           AluOpType.mult, AluOpType.add)
            WIDX16 = per.tile([K2, HALF], I16, tag="widx16")
            nc.vector.tensor_copy(WIDX16[:], FI[:, 1])
            # OM = 0.5 - D = 1 - frac
            OM = per.tile([K2, 2, HALF], F32, tag="om")
            nc.vector.tensor_scalar(OM[:], AYX[:], -1.0, 0.5,
                                    AluOpType.mult, AluOpType.add)
            # FI[:,1] now free: reuse for frac_x = D_x + 0.5
            nc.vector.tensor_scalar(FI[:, 1], AYX[:, 1], 0.5, None, AluOpType.add)
            # pixel-interleaved weight maps [k, row, (pixel, x-corner)] so the
            # per-iteration broadcast DMA is contiguous; the bf16 (w_x0, w_x1)
            # pair of a pixel shares one fp32 container word
            WMI = per.tile([K2, 2, 2 * HALF], BF16, tag="wmi")
            nc.vector.tensor_tensor(WMI[:, 0, 0::2], OM[:, 0], OM[:, 1],
                                    AluOpType.mult)
            nc.vector.tensor_tensor(WMI[:, 0, 1::2], OM[:, 0], FI[:, 1],
                                    AluOpType.mult)
            nc.vector.scalar_tensor_tensor(WMI[:, 1, 0::2], AYX[:, 0], 0.5,
                                           OM[:, 1], AluOpType.add,
                                           AluOpType.mult)
            nc.vector.scalar_tensor_tensor(WMI[:, 1, 1::2], AYX[:, 0], 0.5,
                                           FI[:, 1], AluOpType.add,
                                           AluOpType.mult)

            # ---- DRAM bounce: wrap indices for ap_gather, broadcast weights ----
            nc.sync.dma_start(widx_d[:], WIDX16[:])
            nc.sync.dma_start(wmap_d[:], WMI.bitcast(F32)[:])
            WIDXW = per.tile([128, K2, HALF // 16], I16, tag="widxw")
            widx_r = widx_d.rearrange("k (c s) -> s k c", s=16)
            for g in range(8):
                nc.sync.dma_start(WIDXW[16 * g:16 * (g + 1)], widx_r)

            # ---- main loop: gather, weight, GEMM-accumulate ----
            PSD0 = psp.tile([128, HALF], F32, tag="ps")
            PSD1 = psp.tile([128, HALF], F32, tag="ps")
            PSD = [PSD0, PSD1]
            for kk in range(K2):
                for row in range(2):
                    # contiguous broadcast of the pixel-interleaved pair weights
                    WB = wbp.tile([128, HALF], F32, tag="wb")
                    nc.sync.dma_start(WB[:], wmap_d[kk, row].unsqueeze(0)
                                      .broadcast_to((128, HALF)))
                    # one d=2 gather fetches both channel groups per index
                    G = gtp.tile([128, HALF, 2], F32, tag="g")
                    nc.gpsimd.ap_gather(
                        G[:], XP2[:, EG * row:EG * row + NELEM + 33],
                        WIDXW[:, kk], channels=128, num_elems=NELEM + 33,
                        d=2, num_idxs=HALF)
                    Gb = G.bitcast(BF16)  # [128, HALF, 4]
                    for cg in range(2):
                        T = ttp.tile([128, 2 * HALF], BF16, tag="t")
                        nc.vector.tensor_tensor(
                            T.rearrange("p (i e) -> p i e", e=2)[:],
                            Gb[:, :, 2 * cg:2 * cg + 2],
                            WB.bitcast(BF16).rearrange("p (i e) -> p i e",
                                                       e=2)[:],
                            AluOpType.mult)
                        first = (kk == 0 and row == 0 and cg == 0)
                        last = (kk == K2 - 1 and row == 1 and cg == 1)
                        for oh in range(2):
                            stat = WDEF[:, cg, kk, 128 * oh:128 * (oh + 1)]
                            for par in range(2):
                                Tp = T[:, par::2]
                                for q in range(4):
                                    mov = Tp[:, 512 * q:512 * (q + 1)]
                                    nc.tensor.matmul(
                                        PSD[oh][:, 512 * q:512 * (q + 1)],
                                        stat, mov,
                                        start=(first and par == 0),
                                        stop=(last and par == 1))

            # ---- BN stats + AllReduce ----
            SM = per.tile([128, 48], F32, tag="sm")
            TRASH = ttp.tile([128, 2 * HALF], BF16, tag="t")
            ZERO = SM[:, 40:41]
            EPSAP = SM[:, 41:42]
            nc.vector.memset(ZERO, 0.0)
            nc.vector.memset(EPSAP, float(EPS))
            for oh in range(2):
                nc.vector.tensor_reduce(SM[:, oh:oh + 1], PSD[oh][:],
                                        AX.X, AluOpType.add)
                nc.scalar.activation(TRASH[:, 0:HALF], PSD[oh][:], AF.Square,
                                     bias=ZERO, accum_out=SM[:, 2 + oh:3 + oh])
            nc.sync.dma_start(stats_in_d[:], SM[:, 0:4])
            nc.gpsimd.collective_compute(
                "AllReduce", AluOpType.add,
                replica_groups=[list(range(num_devices))],
                ins=[stats_in_d[:]], outs=[stats_out_d[:]])
            nc.sync.dma_start(SM[:, 8:12], stats_out_d[:])

            for oh in range(2):
                mean = SM[:, 16 + oh:17 + oh]
                ex2 = SM[:, 18 + oh:19 + oh]
                var = SM[:, 20 + oh:21 + oh]
                sd = SM[:, 22 + oh:23 + oh]
                rstd = SM[:, 24 + oh:25 + oh]
                s1 = SM[:, 26 + oh:27 + oh]
                ms = SM[:, 28 + oh:29 + oh]
                s2 = SM[:, 30 + oh:31 + oh]
                nc.vector.tensor_scalar(mean, SM[:, 8 + oh:9 + oh],
                                        1.0 / NTOT, None, AluOpType.mult)
                nc.vector.tensor_scalar(ex2, SM[:, 10 + oh:11 + oh],
                                        1.0 / NTOT, None, AluOpType.mult)
                nc.vector.tensor_tensor(var, mean, mean, AluOpType.mult)
                nc.vector.tensor_tensor(var, ex2, var, AluOpType.subtract)
                nc.scalar.activation(sd, var, AF.Sqrt, bias=EPSAP)
                nc.vector.reciprocal(rstd, sd)
                nc.vector.tensor_tensor(s1, GBt[:, oh, 0:1], rstd, AluOpType.mult)
                nc.vector.tensor_tensor(ms, mean, s1, AluOpType.mult)
                nc.vector.tensor_tensor(s2, GBt[:, oh, 1:2], ms, AluOpType.subtract)
                OUTS = gtp.tile([128, HALF], F32, tag="g")
                nc.scalar.activation(OUTS[:], PSD[oh][:], AF.Relu,
                                     bias=s2, scale=s1)
                nc.sync.dma_start(out_d[128 * oh:128 * (oh + 1), :], OUTS[:])

    nc.compile()
    return nc


def host_inputs(x, w_off, b_off, w_def, gamma, beta):
    """Build the 8 per-core input dicts."""
    x = np.asarray(x, np.float32)
    w_off = np.asarray(w_off, np.float32)
    b_off = np.asarray(b_off, np.float32)
    w_def = np.asarray(w_def, np.float32)
    gamma = np.asarray(gamma, np.float32)
    beta = np.asarray(beta, np.float32)

    # weight stationaries, shared by all cores.
    # woff[cg, c, kk, j]: off-conv stationary for kernel position kk; output
    # column j<9 = tap j's dy channel (2j), j>=9 = tap (j-9)'s dx channel.
    woff = np.zeros((2, 128, K2, 18), np.float32)
    wdef = np.zeros((2, 128, K2, O), np.float32)
    for cg in range(2):
        cs = slice(128 * cg, 128 * (cg + 1))
        for kk in range(K2):
            ky, kx = kk // K, kk % K
            for j in range(K2):
                woff[cg, :, kk, j] = w_off[2 * j, cs, ky, kx]
                woff[cg, :, kk, 9 + j] = w_off[2 * j + 1, cs, ky, kx]
            wdef[cg, :, kk, :] = w_def[:, cs, ky, kx].T
    woff = woff.astype(ml_dtypes.bfloat16)
    wdef = wdef.astype(ml_dtypes.bfloat16)

    gb = np.zeros((2, 128, 2), np.float32)
    gb[0, :, 0], gb[1, :, 0] = gamma[:128], gamma[128:]
    gb[0, :, 1], gb[1, :, 1] = beta[:128], beta[128:]

    in_maps = []
    for core in range(NCORES):
        b, ph = core // 2, core % 2
        # extended zero-padded grid, rolled so ext rows [0..80) cover this
        # core's rows: ext row r corresponds to image row r - 8 + 32*ph
        xe = np.zeros((C, EG, EG), np.float32)
        r_lo, r_hi = 32 * ph - PADE, 32 * ph - PADE + EG
        s_lo, s_hi = max(0, r_lo), min(H, r_hi)
        xe[:, s_lo - r_lo:s_hi - r_lo, PADE:PADE + W] = x[b, :, s_lo:s_hi, :]
        # container pack for ap_gather: element i = (xe[i], xe[i+1])
        xcols = XTROWS + EG
        flatc = np.zeros((C, xcols + 1), ml_dtypes.bfloat16)
        flatc[:, PREB:PREB + EGF] = xe.reshape(C, EGF)
        lo = flatc[:, :xcols].view(np.uint16).astype(np.uint32)
        hi = flatc[:, 1:xcols + 1].view(np.uint16).astype(np.uint32)
        xpf = (lo | (hi << np.uint32(16))).view(np.float32).reshape(2, 128, xcols)
        # d=2 element = (cg0 container, cg1 container)
        xp = np.ascontiguousarray(np.stack([xpf[0], xpf[1]], axis=-1))

        # kb[k, 0, p] = 16 + (ky-1) + h_local(p) + b_off_y ; h_local = p//64 + ...
        # NOTE: the gather/window row coords are *local* to the rolled grid:
        # local row of pixel p is p//64 (0..31), plus the conv sampling is
        # relative; py_local = off_y + (ky-1) + p//64.  The +16 mod-floor bias.
        kb = np.zeros((K2, 2, HALF), np.float32)
        pl = np.arange(HALF, dtype=np.float32)
        hloc = np.floor(pl / W)
        wloc = pl % W
        for kk in range(K2):
            ky, kx = kk // K, kk % K
            kb[kk, 0, :] = 15.5 + (ky - 1) + hloc + b_off[2 * kk]
            kb[kk, 1, :] = 15.5 + (kx - 1) + wloc + b_off[2 * kk + 1]
        m = {"woff": np.asarray(woff), "wdef": np.asarray(wdef),
             "kb": kb, "gb": gb, "xp": xp}
        in_maps.append(m)
    return in_maps


_prog_cache = {}


def _get_prog():
    if "nc" not in _prog_cache:
        _prog_cache["nc"] = build_program(NCORES)
    return _prog_cache["nc"]


def kernel(x, w_off, b_off, w_def, gamma, beta):
    nc = _get_prog()
    in_maps = host_inputs(x, w_off, b_off, w_def, gamma, beta)
    res = run_bass_kernel_spmd(nc, in_maps, core_ids=list(range(NCORES)))
    out = np.zeros((B, O, H, W), np.float32)
    for core in range(NCORES):
        b, ph = core // 2, core % 2
        out[b, :, 32 * ph:32 * (ph + 1), :] = \
            res.results[core]["out"].reshape(O, HROWS, W)
    return out



# revision 13
# speedup vs baseline: 1.2657x; 1.2657x over previous
"""Deformable-conv module (offset conv -> bilinear deform conv -> sync-BN -> ReLU)
as a Trainium2 Bass kernel on 8 NeuronCores.

Sharding: core = (batch b, pixel-half ph).  Each core computes the full
256-channel output for 2048 pixels (32 image rows) of one batch image.
Full C=256 contraction is local, so no partial-sum exchange is needed;
only BN statistics cross cores (one 4KB AllReduce).

Bilinear sampling: x is host-padded into an 80x80 zero-extended grid and
packed as bf16 (value, right-neighbor) pairs in fp32 containers.  One
ap_gather index fetches both x-corners of a row; the row+1 corners come
from the same index into the grid shifted by one row.  Zero padding makes
all out-of-image corners contribute exactly 0, so no validity masks are
needed.  The 4-corner bilinear sum is folded into the deform GEMM's
contraction (each corner stream is a moving operand; PSUM accumulates).
"""
import sys, os, time

sys.path.insert(0, "/opt/trn_rl_repo")

import numpy as np
import ml_dtypes

import concourse.bacc as bacc
import concourse.tile as tile
from concourse import mybir
from concourse import library_config
from concourse.alu_op_type import AluOpType
from concourse.bass_utils import run_bass_kernel_spmd

F32 = mybir.dt.float32
BF16 = mybir.dt.bfloat16
I16 = mybir.dt.int16
AF = mybir.ActivationFunctionType
AX = mybir.AxisListType
MM = mybir.MatmulPerfMode

B, C, H, W, O = 4, 256, 64, 64, 256
K = 3
K2 = 9
EPS = 1e-5
PADE = 8          # extension pad on each side of the image
EG = H + 2 * PADE  # 80: extended grid edge
EGF = EG * EG      # 6400 ext pixels
PREB = 648         # leading zero rows so biased indices need no -648
XTROWS = PREB + EGF  # 7048 rows in the HBM gather table
NELEM = 6967       # gather source rows (covers max biased index 6966)
MAGIC = 12582912.0  # 1.5 * 2**23: fp32 round-to-int trick
HALF = 2048        # pixels per core
HROWS = HALF // W  # 32 image rows per core
NCORES = 8
NTOT = B * H * W   # BN normalization count


def build_program(num_devices=NCORES):
    nc = bacc.Bacc("TRN2", target_bir_lowering=False, debug=False,
                   num_devices=num_devices, num_swdge_queues=4)

    xp_d = nc.dram_tensor("xp", [128, XTROWS + EG, 2], F32,
                          kind="ExternalInput").ap()
    woff_d = nc.dram_tensor("woff", [2, 128, K2, 18], BF16, kind="ExternalInput").ap()
    wdef_d = nc.dram_tensor("wdef", [2, 128, K2, O], BF16, kind="ExternalInput").ap()
    kb_d = nc.dram_tensor("kb", [K2, 2, HALF], F32, kind="ExternalInput").ap()
    gb_d = nc.dram_tensor("gb", [2, 128, 2], F32, kind="ExternalInput").ap()
    out_d = nc.dram_tensor("out", [O, HALF], F32, kind="ExternalOutput").ap()
    # internal DRAM scratch for layout bounces
    widx_d = nc.dram_tensor("widx_s", [K2, HALF], I16).ap()
    wmap_d = nc.dram_tensor("wmap_s", [K2, 2, HALF], F32).ap()
    stats_in_d = nc.dram_tensor("stats_in", [128, 4], F32).ap()
    stats_out_d = nc.dram_tensor("stats_out", [128, 4], F32, addr_space="Shared").ap()

    with tile.TileContext(nc) as tc:
        with tc.tile_pool(name="per", bufs=1) as per, \
             tc.tile_pool(name="wb", bufs=2) as wbp, \
             tc.tile_pool(name="gt", bufs=2) as gtp, \
             tc.tile_pool(name="tt", bufs=2) as ttp, \
             tc.tile_pool(name="ps", bufs=2, space="PSUM") as psp:

            # ---- inputs to SBUF ----
            # d=2 gather table: element i = (cg0 container, cg1 container),
            # each container = bf16 (xe[i], xe[i+1]) pair in an fp32 word.
            # One gather request then fetches both channel groups, halving the
            # Q7 read-request count that rate-limits ap_gather.
            nc.gpsimd.load_library(library_config.ap_gather)
            XP2 = per.tile([128, XTROWS + EG, 2], F32, tag="xp2")
            nc.sync.dma_start(XP2[:], xp_d[:])
            WOFF = per.tile([128, 2, K2, 18], BF16, tag="woff")
            nc.sync.dma_start(WOFF[:, 0], woff_d[0])
            nc.sync.dma_start(WOFF[:, 1], woff_d[1])
            WDEF = per.tile([128, 2, K2, O], BF16, tag="wdef")
            nc.sync.dma_start(WDEF[:, 0], wdef_d[0])
            nc.sync.dma_start(WDEF[:, 1], wdef_d[1])
            GBt = per.tile([128, 2, 2], F32, tag="gb")
            nc.sync.dma_start(GBt[:, 0], gb_d[0])
            nc.sync.dma_start(GBt[:, 1], gb_d[1])

            # [part, 80 ext rows, 80 cols] view for conv windows: stride-4
            # bf16 view into the packed table (bf16 4i+2cg = xe_cg[i])
            xvb = XP2[:, PREB:PREB + EGF].bitcast(BF16)\
                .rearrange("p (r c) e -> p r (c e)", c=EG)
            xv = [xvb[:, :, 2 * cg::4] for cg in range(2)]
            xstep = 1

            # ---- offset conv: two GEMMs (y comps, x comps) ----
            # psum partitions = 9 taps; moving = 32x64 image window, 512-col chunks
            ps_y = psp.tile([K2, HALF], F32, tag="ps")
            ps_x = psp.tile([K2, HALF], F32, tag="ps")
            n_mm = 0
            for comp, pst in ((0, ps_y), (1, ps_x)):
                for cg in range(2):
                    for kk in range(K2):
                        dy, dx = kk // K - 1, kk % K - 1
                        r0 = PADE + dy   # ext row of first window row (ph folded in kb? no: rows differ per core!)
                        # NOTE: per-core row offset handled via kb; but the
                        # window itself must read this core's rows.  The row
                        # base depends on ph which is NOT known at build time,
                        # so we bake it via a per-core DRAM input instead?  No:
                        # all cores run the same program; we make the window
                        # row base a *program constant* = PADE+dy and add the
                        # per-core 32-row offset by shifting the data on host?
                        # Host shifts: xp is the full 80x80 grid; instead the
                        # moving AP below uses rows [PADE+dy+ROWOFF ...] with
                        # ROWOFF supplied via host-rolled kb... -> resolved by
                        # building xp PER CORE with the core's 32-row window
                        # centered: see host prep (xe rolled so that rows
                        # [PADE..PADE+32) are this core's rows).
                        stat = WOFF[:, cg, kk, 9 * comp:9 * comp + 9]
                        for q in range(4):
                            rq = r0 + 8 * q
                            mov = xv[cg][:, rq:rq + 8,
                                         xstep * (PADE + dx):
                                         xstep * (PADE + dx) + xstep * W:xstep]
                            nc.tensor.matmul(
                                pst[:, 512 * q:512 * (q + 1)], stat, mov,
                                start=(cg == 0 and kk == 0),
                                stop=(cg == 1 and kk == K2 - 1))
                            n_mm += 1

            # ---- coordinate math ----
            # A = py + 15.5 (kb holds tap offset + base + 15.5); add conv psums
            AYX = per.tile([K2, 2, HALF], F32, tag="ayx")
            nc.sync.dma_start(AYX[:], kb_d[:])
            nc.vector.tensor_tensor(AYX[:, 0], AYX[:, 0], ps_y[:], AluOpType.add)
            nc.vector.tensor_tensor(AYX[:, 1], AYX[:, 1], ps_x[:], AluOpType.add)
            # FI = round(A) = floor(py) + 16 (fp32 magic-number round)
            FI = per.tile([K2, 2, HALF], F32, tag="fi")
            nc.vector.tensor_scalar(FI[:], AYX[:], MAGIC, -MAGIC,
                                    AluOpType.add, AluOpType.add)
            # D = A - FI in (-0.5, 0.5]; frac = D + 0.5, 1-frac = 0.5 - D
            nc.vector.tensor_tensor(AYX[:], AYX[:], FI[:], AluOpType.subtract)
            # clip FI to [8, 86] (= coord in [-8, 70]), in place
            nc.vector.tensor_scalar(FI[:], FI[:], 8.0, 86.0,
                                    AluOpType.max, AluOpType.min)
            # gather index = 80*ycl + xcl (maps into the PREB-padded grid)
            nc.vector.scalar_tensor_tensor(FI[:, 1], FI[:, 0], 80.0, FI[:, 1],
                                           AluOpType.mult, AluOpType.add)
            # s-major permuted copy: WIDX16[k, 128*s + c] = idx[k, 16*c + s],
            # so the wrapped [16-part, k, c] DRAM reload below reads 256-byte
            # contiguous runs (144 descriptors) instead of one descriptor per
            # 2-byte element (147456 descriptors, ~360us of DMA-engine time).
            WIDX16 = per.tile([K2, HALF], I16, tag="widx16")
            nc.vector.tensor_copy(
                WIDX16.rearrange("k (s c) -> k s c", s=16)[:],
                FI[:, 1].rearrange("k (c s) -> k s c", s=16))
            # OM = 0.5 - D = 1 - frac
            OM = per.tile([K2, 2, HALF], F32, tag="om")
            nc.vector.tensor_scalar(OM[:], AYX[:], -1.0, 0.5,
                                    AluOpType.mult, AluOpType.add)
            # FI[:,1] now free: reuse for frac_x = D_x + 0.5
            nc.vector.tensor_scalar(FI[:, 1], AYX[:, 1], 0.5, None, AluOpType.add)
            # pixel-interleaved weight maps [k, row, (pixel, x-corner)] so the
            # per-iteration broadcast DMA is contiguous; the bf16 (w_x0, w_x1)
            # pair of a pixel shares one fp32 container word
            WMI = per.tile([K2, 2, 2 * HALF], BF16, tag="wmi")
            nc.vector.tensor_tensor(WMI[:, 0, 0::2], OM[:, 0], OM[:, 1],
                                    AluOpType.mult)
            nc.vector.tensor_tensor(WMI[:, 0, 1::2], OM[:, 0], FI[:, 1],
                                    AluOpType.mult)
            nc.vector.scalar_tensor_tensor(WMI[:, 1, 0::2], AYX[:, 0], 0.5,
                                           OM[:, 1], AluOpType.add,
                                           AluOpType.mult)
            nc.vector.scalar_tensor_tensor(WMI[:, 1, 1::2], AYX[:, 0], 0.5,
                                           FI[:, 1], AluOpType.add,
                                           AluOpType.mult)

            # ---- DRAM bounce: wrap indices for ap_gather, broadcast weights ----
            nc.sync.dma_start(widx_d[:], WIDX16[:])
            nc.sync.dma_start(wmap_d[:], WMI.bitcast(F32)[:])
            WIDXW = per.tile([128, K2, HALF // 16], I16, tag="widxw")
            widx_r = widx_d.rearrange("k (s c) -> s k c", s=16)
            for g in range(8):
                nc.sync.dma_start(WIDXW[16 * g:16 * (g + 1)], widx_r)

            # ---- main loop: gather, weight, GEMM-accumulate ----
            PSD0 = psp.tile([128, HALF], F32, tag="ps")
            PSD1 = psp.tile([128, HALF], F32, tag="ps")
            PSD = [PSD0, PSD1]
            for kk in range(K2):
                for row in range(2):
                    # contiguous broadcast of the pixel-interleaved pair weights
                    WB = wbp.tile([128, HALF], F32, tag="wb")
                    nc.sync.dma_start(WB[:], wmap_d[kk, row].unsqueeze(0)
                                      .broadcast_to((128, HALF)))
                    # one d=2 gather fetches both channel groups per index
                    G = gtp.tile([128, HALF, 2], F32, tag="g")
                    nc.gpsimd.ap_gather(
                        G[:], XP2[:, EG * row:EG * row + NELEM + 33],
                        WIDXW[:, kk], channels=128, num_elems=NELEM + 33,
                        d=2, num_idxs=HALF)
                    Gb = G.bitcast(BF16)  # [128, HALF, 4]
                    for cg in range(2):
                        T = ttp.tile([128, 2 * HALF], BF16, tag="t")
                        nc.vector.tensor_tensor(
                            T.rearrange("p (i e) -> p i e", e=2)[:],
                            Gb[:, :, 2 * cg:2 * cg + 2],
                            WB.bitcast(BF16).rearrange("p (i e) -> p i e",
                                                       e=2)[:],
                            AluOpType.mult)
                        first = (kk == 0 and row == 0 and cg == 0)
                        last = (kk == K2 - 1 and row == 1 and cg == 1)
                        for oh in range(2):
                            stat = WDEF[:, cg, kk, 128 * oh:128 * (oh + 1)]
                            for par in range(2):
                                Tp = T[:, par::2]
                                for q in range(4):
                                    mov = Tp[:, 512 * q:512 * (q + 1)]
                                    nc.tensor.matmul(
                                        PSD[oh][:, 512 * q:512 * (q + 1)],
                                        stat, mov,
                                        start=(first and par == 0),
                                        stop=(last and par == 1))

            # ---- BN stats + AllReduce ----
            SM = per.tile([128, 48], F32, tag="sm")
            TRASH = ttp.tile([128, 2 * HALF], BF16, tag="t")
            ZERO = SM[:, 40:41]
            EPSAP = SM[:, 41:42]
            nc.vector.memset(ZERO, 0.0)
            nc.vector.memset(EPSAP, float(EPS))
            for oh in range(2):
                nc.vector.tensor_reduce(SM[:, oh:oh + 1], PSD[oh][:],
                                        AX.X, AluOpType.add)
                nc.scalar.activation(TRASH[:, 0:HALF], PSD[oh][:], AF.Square,
                                     bias=ZERO, accum_out=SM[:, 2 + oh:3 + oh])
            nc.sync.dma_start(stats_in_d[:], SM[:, 0:4])
            nc.gpsimd.collective_compute(
                "AllReduce", AluOpType.add,
                replica_groups=[list(range(num_devices))],
                ins=[stats_in_d[:]], outs=[stats_out_d[:]])
            nc.sync.dma_start(SM[:, 8:12], stats_out_d[:])

            for oh in range(2):
                mean = SM[:, 16 + oh:17 + oh]
                ex2 = SM[:, 18 + oh:19 + oh]
                var = SM[:, 20 + oh:21 + oh]
                sd = SM[:, 22 + oh:23 + oh]
                rstd = SM[:, 24 + oh:25 + oh]
                s1 = SM[:, 26 + oh:27 + oh]
                ms = SM[:, 28 + oh:29 + oh]
                s2 = SM[:, 30 + oh:31 + oh]
                nc.vector.tensor_scalar(mean, SM[:, 8 + oh:9 + oh],
                                        1.0 / NTOT, None, AluOpType.mult)
                nc.vector.tensor_scalar(ex2, SM[:, 10 + oh:11 + oh],
                                        1.0 / NTOT, None, AluOpType.mult)
                nc.vector.tensor_tensor(var, mean, mean, AluOpType.mult)
                nc.vector.tensor_tensor(var, ex2, var, AluOpType.subtract)
                nc.scalar.activation(sd, var, AF.Sqrt, bias=EPSAP)
                nc.vector.reciprocal(rstd, sd)
                nc.vector.tensor_tensor(s1, GBt[:, oh, 0:1], rstd, AluOpType.mult)
                nc.vector.tensor_tensor(ms, mean, s1, AluOpType.mult)
                nc.vector.tensor_tensor(s2, GBt[:, oh, 1:2], ms, AluOpType.subtract)
                OUTS = gtp.tile([128, HALF], F32, tag="g")
                nc.scalar.activation(OUTS[:], PSD[oh][:], AF.Relu,
                                     bias=s2, scale=s1)
                nc.sync.dma_start(out_d[128 * oh:128 * (oh + 1), :], OUTS[:])

    nc.compile()
    return nc


def host_inputs(x, w_off, b_off, w_def, gamma, beta):
    """Build the 8 per-core input dicts."""
    x = np.asarray(x, np.float32)
    w_off = np.asarray(w_off, np.float32)
    b_off = np.asarray(b_off, np.float32)
    w_def = np.asarray(w_def, np.float32)
    gamma = np.asarray(gamma, np.float32)
    beta = np.asarray(beta, np.float32)

    # weight stationaries, shared by all cores.
    # woff[cg, c, kk, j]: off-conv stationary for kernel position kk; output
    # column j<9 = tap j's dy channel (2j), j>=9 = tap (j-9)'s dx channel.
    woff = np.zeros((2, 128, K2, 18), np.float32)
    wdef = np.zeros((2, 128, K2, O), np.float32)
    for cg in range(2):
        cs = slice(128 * cg, 128 * (cg + 1))
        for kk in range(K2):
            ky, kx = kk // K, kk % K
            for j in range(K2):
                woff[cg, :, kk, j] = w_off[2 * j, cs, ky, kx]
                woff[cg, :, kk, 9 + j] = w_off[2 * j + 1, cs, ky, kx]
            wdef[cg, :, kk, :] = w_def[:, cs, ky, kx].T
    woff = woff.astype(ml_dtypes.bfloat16)
    wdef = wdef.astype(ml_dtypes.bfloat16)

    gb = np.zeros((2, 128, 2), np.float32)
    gb[0, :, 0], gb[1, :, 0] = gamma[:128], gamma[128:]
    gb[0, :, 1], gb[1, :, 1] = beta[:128], beta[128:]

    in_maps = []
    for core in range(NCORES):
        b, ph = core // 2, core % 2
        # extended zero-padded grid, rolled so ext rows [0..80) cover this
        # core's rows: ext row r corresponds to image row r - 8 + 32*ph
        xe = np.zeros((C, EG, EG), np.float32)
        r_lo, r_hi = 32 * ph - PADE, 32 * ph - PADE + EG
        s_lo, s_hi = max(0, r_lo), min(H, r_hi)
        xe[:, s_lo - r_lo:s_hi - r_lo, PADE:PADE + W] = x[b, :, s_lo:s_hi, :]
        # container pack for ap_gather: element i = (xe[i], xe[i+1])
        xcols = XTROWS + EG
        flatc = np.zeros((C, xcols + 1), ml_dtypes.bfloat16)
        flatc[:, PREB:PREB + EGF] = xe.reshape(C, EGF)
        lo = flatc[:, :xcols].view(np.uint16).astype(np.uint32)
        hi = flatc[:, 1:xcols + 1].view(np.uint16).astype(np.uint32)
        xpf = (lo | (hi << np.uint32(16))).view(np.float32).reshape(2, 128, xcols)
        # d=2 element = (cg0 container, cg1 container)
        xp = np.ascontiguousarray(np.stack([xpf[0], xpf[1]], axis=-1))

        # kb[k, 0, p] = 16 + (ky-1) + h_local(p) + b_off_y ; h_local = p//64 + ...
        # NOTE: the gather/window row coords are *local* to the rolled grid:
        # local row of pixel p is p//64 (0..31), plus the conv sampling is
        # relative; py_local = off_y + (ky-1) + p//64.  The +16 mod-floor bias.
        kb = np.zeros((K2, 2, HALF), np.float32)
        pl = np.arange(HALF, dtype=np.float32)
        hloc = np.floor(pl / W)
        wloc = pl % W
        for kk in range(K2):
            ky, kx = kk // K, kk % K
            kb[kk, 0, :] = 15.5 + (ky - 1) + hloc + b_off[2 * kk]
            kb[kk, 1, :] = 15.5 + (kx - 1) + wloc + b_off[2 * kk + 1]
        m = {"woff": np.asarray(woff), "wdef": np.asarray(wdef),
             "kb": kb, "gb": gb, "xp": xp}
        in_maps.append(m)
    return in_maps


_prog_cache = {}


def _get_prog():
    if "nc" not in _prog_cache:
        _prog_cache["nc"] = build_program(NCORES)
    return _prog_cache["nc"]


def kernel(x, w_off, b_off, w_def, gamma, beta):
    nc = _get_prog()
    in_maps = host_inputs(x, w_off, b_off, w_def, gamma, beta)
    res = run_bass_kernel_spmd(nc, in_maps, core_ids=list(range(NCORES)))
    out = np.zeros((B, O, H, W), np.float32)
    for core in range(NCORES):
        b, ph = core // 2, core % 2
        out[b, :, 32 * ph:32 * (ph + 1), :] = \
            res.results[core]["out"].reshape(O, HROWS, W)
    return out



# revision 15
# speedup vs baseline: 4.3842x; 3.4639x over previous
"""Deformable-conv module (offset conv -> bilinear deform conv -> sync-BN -> ReLU)
as a Trainium2 Bass kernel on 8 NeuronCores.

Sharding: core = (batch b, pixel-half ph).  Each core computes the full
256-channel output for 2048 pixels (32 image rows) of one batch image.
Full C=256 contraction is local; only BN statistics cross cores (one 4KB
AllReduce).

Bilinear sampling via SDMA dma_gather(transpose=True): a host-built HBM
table holds, per extended-grid position i, a 2KB row with all four
bilinear corners x (value, x+1, row+1, row+1/x+1) for both 128-channel
groups.  One 2048-index gather per kernel tap lands the data directly
channel-on-partition ([128, 8, 2048] bf16) through the xbar-transpose
spray, at SDMA bandwidth instead of the Q7 FIFO path.  The 4-corner
bilinear weights are applied with two in-place DVE multiplies and the
corner sum is folded into the deform GEMM's PSUM accumulation.
"""
import sys, os, time

sys.path.insert(0, "/opt/trn_rl_repo")

import numpy as np
import ml_dtypes

import concourse.bacc as bacc
import concourse.tile as tile
from concourse import mybir
from concourse import library_config
from concourse.alu_op_type import AluOpType
from concourse.bass_utils import run_bass_kernel_spmd

F32 = mybir.dt.float32
BF16 = mybir.dt.bfloat16
I16 = mybir.dt.int16
AF = mybir.ActivationFunctionType
AX = mybir.AxisListType

B, C, H, W, O = 4, 256, 64, 64, 256
K = 3
K2 = 9
EPS = 1e-5
PADE = 8           # extension pad on each side of the image
EG = H + 2 * PADE  # 80: extended grid edge
NPOS = EG * EG     # 6400 grid positions (gather table rows)
MAGIC = 12582912.0  # 1.5 * 2**23: fp32 round-to-int trick
HALF = 2048        # pixels per core
HROWS = HALF // W  # 32 image rows per core
NCORES = 8
NTOT = B * H * W   # BN normalization count


def build_program(num_devices=NCORES):
    nc = bacc.Bacc("TRN2", target_bir_lowering=False, debug=False,
                   num_devices=num_devices, num_swdge_queues=4)

    gt_d = nc.dram_tensor("gt", [NPOS, 1024], BF16, kind="ExternalInput").ap()
    xc_d = nc.dram_tensor("xc", [2, 128, 34, 66], BF16, kind="ExternalInput").ap()
    woff_d = nc.dram_tensor("woff", [2, 128, K2, 18], BF16, kind="ExternalInput").ap()
    wdef_d = nc.dram_tensor("wdef", [2, 128, K2, O], BF16, kind="ExternalInput").ap()
    kb_d = nc.dram_tensor("kb", [K2, 2, HALF], F32, kind="ExternalInput").ap()
    gb_d = nc.dram_tensor("gb", [2, 128, 2], F32, kind="ExternalInput").ap()
    out_d = nc.dram_tensor("out", [O, HALF], F32, kind="ExternalOutput").ap()
    # internal DRAM scratch for layout bounces
    widx_d = nc.dram_tensor("widx_s", [K2, HALF], I16).ap()
    wmap_d = nc.dram_tensor("wmap_s", [K2, 4, HALF], BF16).ap()
    stats_in_d = nc.dram_tensor("stats_in", [128, 4], F32).ap()
    stats_out_d = nc.dram_tensor("stats_out", [128, 4], F32, addr_space="Shared").ap()

    with tile.TileContext(nc) as tc:
        with tc.tile_pool(name="per", bufs=1) as per, \
             tc.tile_pool(name="wb", bufs=2) as wbp, \
             tc.tile_pool(name="gt", bufs=2) as gtp, \
             tc.tile_pool(name="ix", bufs=2) as ixp, \
             tc.tile_pool(name="ot", bufs=2) as otp, \
             tc.tile_pool(name="ps", bufs=2, space="PSUM") as psp:

            nc.gpsimd.load_library(library_config.mlp)

            # ---- inputs to SBUF ----
            XC = per.tile([128, 2, 34, 66], BF16, tag="xc")
            nc.sync.dma_start(XC[:, 0], xc_d[0])
            nc.sync.dma_start(XC[:, 1], xc_d[1])
            WOFF = per.tile([128, 2, K2, 18], BF16, tag="woff")
            nc.sync.dma_start(WOFF[:, 0], woff_d[0])
            nc.sync.dma_start(WOFF[:, 1], woff_d[1])
            WDEF = per.tile([128, 2, K2, O], BF16, tag="wdef")
            nc.sync.dma_start(WDEF[:, 0], wdef_d[0])
            nc.sync.dma_start(WDEF[:, 1], wdef_d[1])
            GBt = per.tile([128, 2, 2], F32, tag="gb")
            nc.sync.dma_start(GBt[:, 0], gb_d[0])
            nc.sync.dma_start(GBt[:, 1], gb_d[1])

            # ---- offset conv: two GEMMs (y comps, x comps) ----
            # psum partitions = 9 taps; moving = 8x64 padded image window
            ps_y = psp.tile([K2, HALF], F32, tag="ps")
            ps_x = psp.tile([K2, HALF], F32, tag="ps")
            for comp, pst in ((0, ps_y), (1, ps_x)):
                for cg in range(2):
                    for kk in range(K2):
                        dy, dx = kk // K - 1, kk % K - 1
                        stat = WOFF[:, cg, kk, 9 * comp:9 * comp + 9]
                        for q in range(4):
                            mov = XC[:, cg, 8 * q + dy + 1:8 * q + dy + 9,
                                     dx + 1:dx + 65]
                            nc.tensor.matmul(
                                pst[:, 512 * q:512 * (q + 1)], stat, mov,
                                start=(cg == 0 and kk == 0),
                                stop=(cg == 1 and kk == K2 - 1))

            # ---- coordinate math ----
            # A = py_ext - 0.5 biased (kb holds tap offset + base + 7.5)
            AYX = per.tile([K2, 2, HALF], F32, tag="ayx")
            nc.sync.dma_start(AYX[:], kb_d[:])
            nc.vector.tensor_tensor(AYX[:, 0], AYX[:, 0], ps_y[:], AluOpType.add)
            nc.vector.tensor_tensor(AYX[:, 1], AYX[:, 1], ps_x[:], AluOpType.add)
            # FI = round(A) = floor(py) + 8 (fp32 magic-number round)
            FI = per.tile([K2, 2, HALF], F32, tag="fi")
            nc.vector.tensor_scalar(FI[:], AYX[:], MAGIC, -MAGIC,
                                    AluOpType.add, AluOpType.add)
            # D = A - FI in (-0.5, 0.5]
            nc.vector.tensor_tensor(AYX[:], AYX[:], FI[:], AluOpType.subtract)
            # clip FI to [0, 78] ext-grid coords, in place
            nc.vector.tensor_scalar(FI[:], FI[:], 0.0, 78.0,
                                    AluOpType.max, AluOpType.min)
            # gather index = 80*ycl + xcl
            nc.vector.scalar_tensor_tensor(FI[:, 1], FI[:, 0], float(EG), FI[:, 1],
                                           AluOpType.mult, AluOpType.add)
            # s-major permuted copy: WIDX16[k, 128*s + c] = idx[k, 16*c + s]
            # so the wrapped [16-part, k, c] DRAM reload has 256B-contiguous runs
            WIDX16 = per.tile([K2, HALF], I16, tag="widx16")
            nc.vector.tensor_copy(
                WIDX16.rearrange("k (s c) -> k s c", s=16)[:],
                FI[:, 1].rearrange("k (c s) -> k s c", s=16))
            # OM = 0.5 - D = 1 - frac (both comps); FI is free now
            nc.vector.tensor_scalar(FI[:], AYX[:], -1.0, 0.5,
                                    AluOpType.mult, AluOpType.add)
            # corner weight maps [k, corner(2*dy+dx), pixel] in bf16
            WMI = per.tile([K2, 4, HALF], BF16, tag="wmi")
            nc.vector.tensor_tensor(WMI[:, 0], FI[:, 0], FI[:, 1],
                                    AluOpType.mult)
            nc.vector.scalar_tensor_tensor(WMI[:, 1], AYX[:, 1], 0.5, FI[:, 0],
                                           AluOpType.add, AluOpType.mult)
            nc.vector.scalar_tensor_tensor(WMI[:, 2], AYX[:, 0], 0.5, FI[:, 1],
                                           AluOpType.add, AluOpType.mult)
            nc.vector.tensor_scalar(AYX[:, 1], AYX[:, 1], 0.5, None,
                                    AluOpType.add)
            nc.vector.scalar_tensor_tensor(WMI[:, 3], AYX[:, 0], 0.5, AYX[:, 1],
                                           AluOpType.add, AluOpType.mult)

            # ---- DRAM bounce: wrapped idxs for dma_gather, weight maps ----
            nc.sync.dma_start(widx_d[:], WIDX16[:])
            nc.sync.dma_start(wmap_d[:], WMI[:])
            WIDXW = per.tile([128, K2, HALF // 16], I16, tag="widxw")
            widx_r = widx_d.rearrange("k (s c) -> s k c", s=16)
            for g in range(8):
                nc.sync.dma_start(WIDXW[16 * g:16 * (g + 1)], widx_r)

            # ---- main loop: gather, weight, GEMM-accumulate ----
            PSD0 = psp.tile([128, HALF], F32, tag="ps")
            PSD1 = psp.tile([128, HALF], F32, tag="ps")
            PSD = [PSD0, PSD1]
            for kk in range(K2):
                # broadcast of the 4 corner-weight rows for this tap
                WMB = wbp.tile([128, 4, HALF], BF16, tag="wb")
                nc.sync.dma_start(WMB[:], wmap_d[kk].unsqueeze(0)
                                  .broadcast_to((128, 4, HALF)))
                # SDMA gathers fetch all 4 corners x both channel groups:
                # G[p, qv, 4*cg + (2*dy+dx), i] = table row idx block.
                # 512-idx batches: larger ones exceed the 64-descriptor
                # per-lane packet cap and hang the DMA engines.
                G = gtp.tile([128, 4, 8, 512], BF16, tag="g")
                for qv in range(4):
                    nc.gpsimd.dma_gather(
                        G[:, qv], gt_d[:],
                        WIDXW[:, kk, 32 * qv:32 * (qv + 1)], 512, 512,
                        1024, transpose=True, queue_num=0)
                # apply bilinear corner weights (in place, per channel group)
                for qv in range(4):
                    for cg in range(2):
                        nc.vector.tensor_tensor(
                            G[:, qv, 4 * cg:4 * cg + 4],
                            G[:, qv, 4 * cg:4 * cg + 4],
                            WMB[:, :, 512 * qv:512 * (qv + 1)],
                            AluOpType.mult)
                first = (kk == 0)
                last = (kk == K2 - 1)
                for cg in range(2):
                    for oh in range(2):
                        stat = WDEF[:, cg, kk, 128 * oh:128 * (oh + 1)]
                        for corner in range(4):
                            for qv in range(4):
                                nc.tensor.matmul(
                                    PSD[oh][:, 512 * qv:512 * (qv + 1)],
                                    stat, G[:, qv, 4 * cg + corner],
                                    start=(first and cg == 0 and corner == 0),
                                    stop=(last and cg == 1 and corner == 3))

            # ---- BN stats + AllReduce ----
            SM = per.tile([128, 48], F32, tag="sm")
            TRASH = per.tile([128, HALF], BF16, tag="trash")
            ZERO = SM[:, 40:41]
            EPSAP = SM[:, 41:42]
            nc.vector.memset(ZERO, 0.0)
            nc.vector.memset(EPSAP, float(EPS))
            for oh in range(2):
                nc.vector.tensor_reduce(SM[:, oh:oh + 1], PSD[oh][:],
                                        AX.X, AluOpType.add)
                nc.scalar.activation(TRASH[:], PSD[oh][:], AF.Square,
                                     bias=ZERO, accum_out=SM[:, 2 + oh:3 + oh])
            nc.sync.dma_start(stats_in_d[:], SM[:, 0:4])
            nc.gpsimd.collective_compute(
                "AllReduce", AluOpType.add,
                replica_groups=[list(range(num_devices))],
                ins=[stats_in_d[:]], outs=[stats_out_d[:]])
            nc.sync.dma_start(SM[:, 8:12], stats_out_d[:])

            for oh in range(2):
                mean = SM[:, 16 + oh:17 + oh]
                ex2 = SM[:, 18 + oh:19 + oh]
                var = SM[:, 20 + oh:21 + oh]
                sd = SM[:, 22 + oh:23 + oh]
                rstd = SM[:, 24 + oh:25 + oh]
                s1 = SM[:, 26 + oh:27 + oh]
                ms = SM[:, 28 + oh:29 + oh]
                s2 = SM[:, 30 + oh:31 + oh]
                nc.vector.tensor_scalar(mean, SM[:, 8 + oh:9 + oh],
                                        1.0 / NTOT, None, AluOpType.mult)
                nc.vector.tensor_scalar(ex2, SM[:, 10 + oh:11 + oh],
                                        1.0 / NTOT, None, AluOpType.mult)
                nc.vector.tensor_tensor(var, mean, mean, AluOpType.mult)
                nc.vector.tensor_tensor(var, ex2, var, AluOpType.subtract)
                nc.scalar.activation(sd, var, AF.Sqrt, bias=EPSAP)
                nc.vector.reciprocal(rstd, sd)
                nc.vector.tensor_tensor(s1, GBt[:, oh, 0:1], rstd, AluOpType.mult)
                nc.vector.tensor_tensor(ms, mean, s1, AluOpType.mult)
                nc.vector.tensor_tensor(s2, GBt[:, oh, 1:2], ms, AluOpType.subtract)
                OUTS = otp.tile([128, HALF], F32, tag="o")
                nc.scalar.activation(OUTS[:], PSD[oh][:], AF.Relu,
                                     bias=s2, scale=s1)
                nc.sync.dma_start(out_d[128 * oh:128 * (oh + 1), :], OUTS[:])

    nc.compile()
    return nc


def host_inputs(x, w_off, b_off, w_def, gamma, beta):
    """Build the 8 per-core input dicts."""
    x = np.asarray(x, np.float32)
    w_off = np.asarray(w_off, np.float32)
    b_off = np.asarray(b_off, np.float32)
    w_def = np.asarray(w_def, np.float32)
    gamma = np.asarray(gamma, np.float32)
    beta = np.asarray(beta, np.float32)

    # weight stationaries, shared by all cores.
    woff = np.zeros((2, 128, K2, 18), np.float32)
    wdef = np.zeros((2, 128, K2, O), np.float32)
    for cg in range(2):
        cs = slice(128 * cg, 128 * (cg + 1))
        for kk in range(K2):
            ky, kx = kk // K, kk % K
            for j in range(K2):
                woff[cg, :, kk, j] = w_off[2 * j, cs, ky, kx]
                woff[cg, :, kk, 9 + j] = w_off[2 * j + 1, cs, ky, kx]
            wdef[cg, :, kk, :] = w_def[:, cs, ky, kx].T
    woff = np.asarray(woff.astype(ml_dtypes.bfloat16))
    wdef = np.asarray(wdef.astype(ml_dtypes.bfloat16))

    gb = np.zeros((2, 128, 2), np.float32)
    gb[0, :, 0], gb[1, :, 0] = gamma[:128], gamma[128:]
    gb[0, :, 1], gb[1, :, 1] = beta[:128], beta[128:]

    pl = np.arange(HALF, dtype=np.float32)
    hloc = np.floor(pl / W)
    wloc = pl % W
    kb = np.zeros((K2, 2, HALF), np.float32)
    for kk in range(K2):
        ky, kx = kk // K, kk % K
        kb[kk, 0, :] = 7.5 + (ky - 1) + hloc + b_off[2 * kk]
        kb[kk, 1, :] = 7.5 + (kx - 1) + wloc + b_off[2 * kk + 1]

    in_maps = []
    for core in range(NCORES):
        b, ph = core // 2, core % 2
        # gather table: 81x81 zero-extended channel-last grid; ext row r
        # corresponds to image row r - 8 + 32*ph, ext col c to image col c - 8
        E = np.zeros((81, 81, 256), np.float32)
        a_lo, a_hi = 32 * ph - 8, 32 * ph - 8 + 81
        sa_lo, sa_hi = max(0, a_lo), min(H, a_hi)
        E[sa_lo - a_lo:sa_hi - a_lo, 8:8 + W, :] = \
            x[b].transpose(1, 2, 0)[sa_lo:sa_hi, :, :]
        Eb = E.astype(ml_dtypes.bfloat16)
        blocks = []
        for cg in range(2):
            cs = slice(128 * cg, 128 * (cg + 1))
            for dy in (0, 1):
                for dx in (0, 1):
                    blocks.append(Eb[dy:dy + EG, dx:dx + EG, cs])
        gt = np.ascontiguousarray(
            np.concatenate(blocks, axis=2).reshape(NPOS, 1024))

        # conv input: rows 32*ph-1 .. 32*ph+32, cols -1..64, zero padded
        r_lo, r_hi = 32 * ph - 1, 32 * ph + 33
        s_lo, s_hi = max(0, r_lo), min(H, r_hi)
        xcv = np.zeros((C, 34, 66), np.float32)
        xcv[:, s_lo - r_lo:s_hi - r_lo, 1:65] = x[b][:, s_lo:s_hi, :]
        xc = np.ascontiguousarray(
            np.stack([xcv[:128], xcv[128:]]).astype(ml_dtypes.bfloat16))

        m = {"woff": woff, "wdef": wdef, "kb": kb, "gb": gb,
             "gt": gt, "xc": xc}
        in_maps.append(m)
    return in_maps


_prog_cache = {}


def _get_prog():
    if "nc" not in _prog_cache:
        _prog_cache["nc"] = build_program(NCORES)
    return _prog_cache["nc"]


def kernel(x, w_off, b_off, w_def, gamma, beta):
    nc = _get_prog()
    in_maps = host_inputs(x, w_off, b_off, w_def, gamma, beta)
    res = run_bass_kernel_spmd(nc, in_maps, core_ids=list(range(NCORES)))
    out = np.zeros((B, O, H, W), np.float32)
    for core in range(NCORES):
        b, ph = core // 2, core % 2
        out[b, :, 32 * ph:32 * (ph + 1), :] = \
            res.results[core]["out"].reshape(O, HROWS, W)
    return out


# revision 16
# speedup vs baseline: 4.7005x; 1.0722x over previous
"""Deformable-conv module (offset conv -> bilinear deform conv -> sync-BN -> ReLU)
as a Trainium2 Bass kernel on 8 NeuronCores.

Sharding: core = (batch b, pixel-half ph).  Each core computes the full
256-channel output for 2048 pixels (32 image rows) of one batch image.
Full C=256 contraction is local; only BN statistics cross cores (one 4KB
AllReduce).

Bilinear sampling via SDMA dma_gather(transpose=True): a host-built HBM
table holds, per extended-grid position i, a 2KB row with all four
bilinear corners x (value, x+1, row+1, row+1/x+1) for both 128-channel
groups.  One 2048-index gather per kernel tap lands the data directly
channel-on-partition ([128, 8, 2048] bf16) through the xbar-transpose
spray, at SDMA bandwidth instead of the Q7 FIFO path.  The 4-corner
bilinear weights are applied with two in-place DVE multiplies and the
corner sum is folded into the deform GEMM's PSUM accumulation.
"""
import sys, os, time

sys.path.insert(0, "/opt/trn_rl_repo")

import numpy as np
import ml_dtypes

import concourse.bacc as bacc
import concourse.tile as tile
from concourse import mybir
from concourse import library_config
from concourse.alu_op_type import AluOpType
from concourse.bass_utils import run_bass_kernel_spmd

F32 = mybir.dt.float32
BF16 = mybir.dt.bfloat16
I16 = mybir.dt.int16
AF = mybir.ActivationFunctionType
AX = mybir.AxisListType

B, C, H, W, O = 4, 256, 64, 64, 256
K = 3
K2 = 9
EPS = 1e-5
PADE = 8           # extension pad on each side of the image
EG = H + 2 * PADE  # 80: extended grid edge
NPOS = EG * EG     # 6400 grid positions (gather table rows)
MAGIC = 12582912.0  # 1.5 * 2**23: fp32 round-to-int trick
HALF = 2048        # pixels per core
HROWS = HALF // W  # 32 image rows per core
NCORES = 8
NTOT = B * H * W   # BN normalization count


def build_program(num_devices=NCORES):
    nc = bacc.Bacc("TRN2", target_bir_lowering=False, debug=False,
                   num_devices=num_devices, num_swdge_queues=4)

    gt_d = nc.dram_tensor("gt", [NPOS, 1024], BF16, kind="ExternalInput").ap()
    xc_d = nc.dram_tensor("xc", [2, 128, 34, 66], BF16, kind="ExternalInput").ap()
    woff_d = nc.dram_tensor("woff", [2, 128, K2, 18], BF16, kind="ExternalInput").ap()
    wdef_d = nc.dram_tensor("wdef", [2, 128, K2, O], BF16, kind="ExternalInput").ap()
    kb_d = nc.dram_tensor("kb", [K2, 2, HALF], F32, kind="ExternalInput").ap()
    gb_d = nc.dram_tensor("gb", [2, 128, 2], F32, kind="ExternalInput").ap()
    out_d = nc.dram_tensor("out", [O, HALF], F32, kind="ExternalOutput").ap()
    # internal DRAM scratch for layout bounces
    widx_d = nc.dram_tensor("widx_s", [K2, HALF], I16).ap()
    wmap_d = nc.dram_tensor("wmap_s", [K2, 4, HALF], BF16).ap()
    stats_in_d = nc.dram_tensor("stats_in", [128, 4], F32).ap()
    stats_out_d = nc.dram_tensor("stats_out", [128, 4], F32, addr_space="Shared").ap()

    with tile.TileContext(nc) as tc:
        with tc.tile_pool(name="per", bufs=1) as per, \
             tc.tile_pool(name="wb", bufs=2) as wbp, \
             tc.tile_pool(name="gt", bufs=2) as gtp, \
             tc.tile_pool(name="ix", bufs=2) as ixp, \
             tc.tile_pool(name="ot", bufs=2) as otp, \
             tc.tile_pool(name="ps", bufs=2, space="PSUM") as psp:

            nc.gpsimd.load_library(library_config.mlp)

            # ---- inputs to SBUF ----
            XC = per.tile([128, 2, 34, 66], BF16, tag="xc")
            nc.sync.dma_start(XC[:, 0], xc_d[0])
            nc.sync.dma_start(XC[:, 1], xc_d[1])
            WOFF = per.tile([128, 2, K2, 18], BF16, tag="woff")
            nc.sync.dma_start(WOFF[:, 0], woff_d[0])
            nc.sync.dma_start(WOFF[:, 1], woff_d[1])
            WDEF = per.tile([128, 2, K2, O], BF16, tag="wdef")
            nc.sync.dma_start(WDEF[:, 0], wdef_d[0])
            nc.sync.dma_start(WDEF[:, 1], wdef_d[1])
            GBt = per.tile([128, 2, 2], F32, tag="gb")
            nc.sync.dma_start(GBt[:, 0], gb_d[0])
            nc.sync.dma_start(GBt[:, 1], gb_d[1])

            # ---- offset conv: two GEMMs (y comps, x comps) ----
            # psum partitions = 9 taps; moving = 8x64 padded image window
            ps_y = psp.tile([K2, HALF], F32, tag="ps")
            ps_x = psp.tile([K2, HALF], F32, tag="ps")
            for comp, pst in ((0, ps_y), (1, ps_x)):
                for cg in range(2):
                    for kk in range(K2):
                        dy, dx = kk // K - 1, kk % K - 1
                        stat = WOFF[:, cg, kk, 9 * comp:9 * comp + 9]
                        for q in range(4):
                            mov = XC[:, cg, 8 * q + dy + 1:8 * q + dy + 9,
                                     dx + 1:dx + 65]
                            nc.tensor.matmul(
                                pst[:, 512 * q:512 * (q + 1)], stat, mov,
                                start=(cg == 0 and kk == 0),
                                stop=(cg == 1 and kk == K2 - 1))

            # ---- coordinate math ----
            # A = py_ext - 0.5 biased (kb holds tap offset + base + 7.5)
            AYX = per.tile([K2, 2, HALF], F32, tag="ayx")
            nc.sync.dma_start(AYX[:], kb_d[:])
            nc.vector.tensor_tensor(AYX[:, 0], AYX[:, 0], ps_y[:], AluOpType.add)
            nc.vector.tensor_tensor(AYX[:, 1], AYX[:, 1], ps_x[:], AluOpType.add)
            # FI = round(A) = floor(py) + 8 (fp32 magic-number round)
            FI = per.tile([K2, 2, HALF], F32, tag="fi")
            nc.vector.tensor_scalar(FI[:], AYX[:], MAGIC, -MAGIC,
                                    AluOpType.add, AluOpType.add)
            # D = A - FI in (-0.5, 0.5]
            nc.vector.tensor_tensor(AYX[:], AYX[:], FI[:], AluOpType.subtract)
            # clip FI to [0, 78] ext-grid coords, in place
            nc.vector.tensor_scalar(FI[:], FI[:], 0.0, 78.0,
                                    AluOpType.max, AluOpType.min)
            # gather index = 80*ycl + xcl
            nc.vector.scalar_tensor_tensor(FI[:, 1], FI[:, 0], float(EG), FI[:, 1],
                                           AluOpType.mult, AluOpType.add)
            # s-major permuted copy: WIDX16[k, 128*s + c] = idx[k, 16*c + s]
            # so the wrapped [16-part, k, c] DRAM reload has 256B-contiguous runs
            WIDX16 = per.tile([K2, HALF], I16, tag="widx16")
            nc.vector.tensor_copy(
                WIDX16.rearrange("k (s c) -> k s c", s=16)[:],
                FI[:, 1].rearrange("k (c s) -> k s c", s=16))
            # OM = 0.5 - D = 1 - frac (both comps); FI is free now
            nc.vector.tensor_scalar(FI[:], AYX[:], -1.0, 0.5,
                                    AluOpType.mult, AluOpType.add)
            # corner weight maps [k, corner(2*dy+dx), pixel] in bf16
            WMI = per.tile([K2, 4, HALF], BF16, tag="wmi")
            nc.vector.tensor_tensor(WMI[:, 0], FI[:, 0], FI[:, 1],
                                    AluOpType.mult)
            nc.vector.scalar_tensor_tensor(WMI[:, 1], AYX[:, 1], 0.5, FI[:, 0],
                                           AluOpType.add, AluOpType.mult)
            nc.vector.scalar_tensor_tensor(WMI[:, 2], AYX[:, 0], 0.5, FI[:, 1],
                                           AluOpType.add, AluOpType.mult)
            nc.vector.tensor_scalar(AYX[:, 1], AYX[:, 1], 0.5, None,
                                    AluOpType.add)
            nc.vector.scalar_tensor_tensor(WMI[:, 3], AYX[:, 0], 0.5, AYX[:, 1],
                                           AluOpType.add, AluOpType.mult)

            # ---- DRAM bounce: wrapped idxs for dma_gather, weight maps ----
            nc.sync.dma_start(widx_d[:], WIDX16[:])
            nc.sync.dma_start(wmap_d[:], WMI[:])
            WIDXW = per.tile([128, K2, HALF // 16], I16, tag="widxw")
            widx_r = widx_d.rearrange("k (s c) -> s k c", s=16)
            for g in range(8):
                nc.sync.dma_start(WIDXW[16 * g:16 * (g + 1)], widx_r)

            # ---- main loop: gather, weight, GEMM-accumulate ----
            PSD0 = psp.tile([128, HALF], F32, tag="ps")
            PSD1 = psp.tile([128, HALF], F32, tag="ps")
            PSD = [PSD0, PSD1]
            for kk in range(K2):
                # broadcast of the 4 corner-weight rows for this tap
                WMB = wbp.tile([128, 4, HALF], BF16, tag="wb")
                nc.sync.dma_start(WMB[:], wmap_d[kk].unsqueeze(0)
                                  .broadcast_to((128, 4, HALF)))
                # SDMA gathers fetch all 4 corners x both channel groups:
                # G[p, qv, 4*cg + (2*dy+dx), i] = table row idx block.
                # 512-idx batches: larger ones exceed the 64-descriptor
                # per-lane packet cap and hang the DMA engines.
                G = gtp.tile([128, 4, 8, 512], BF16, tag="g")
                for qv in range(4):
                    nc.gpsimd.dma_gather(
                        G[:, qv], gt_d[:],
                        WIDXW[:, kk, 32 * qv:32 * (qv + 1)], 512, 512,
                        1024, transpose=True, queue_num=qv % 2)
                # apply bilinear corner weights (in place, per channel group)
                for qv in range(4):
                    for cg in range(2):
                        nc.vector.tensor_tensor(
                            G[:, qv, 4 * cg:4 * cg + 4],
                            G[:, qv, 4 * cg:4 * cg + 4],
                            WMB[:, :, 512 * qv:512 * (qv + 1)],
                            AluOpType.mult)
                first = (kk == 0)
                last = (kk == K2 - 1)
                for cg in range(2):
                    for oh in range(2):
                        stat = WDEF[:, cg, kk, 128 * oh:128 * (oh + 1)]
                        for corner in range(4):
                            for qv in range(4):
                                nc.tensor.matmul(
                                    PSD[oh][:, 512 * qv:512 * (qv + 1)],
                                    stat, G[:, qv, 4 * cg + corner],
                                    start=(first and cg == 0 and corner == 0),
                                    stop=(last and cg == 1 and corner == 3))

            # ---- BN stats + AllReduce ----
            SM = per.tile([128, 48], F32, tag="sm")
            TRASH = per.tile([128, HALF], BF16, tag="trash")
            ZERO = SM[:, 40:41]
            EPSAP = SM[:, 41:42]
            nc.vector.memset(ZERO, 0.0)
            nc.vector.memset(EPSAP, float(EPS))
            for oh in range(2):
                nc.vector.tensor_reduce(SM[:, oh:oh + 1], PSD[oh][:],
                                        AX.X, AluOpType.add)
                nc.scalar.activation(TRASH[:], PSD[oh][:], AF.Square,
                                     bias=ZERO, accum_out=SM[:, 2 + oh:3 + oh])
            nc.sync.dma_start(stats_in_d[:], SM[:, 0:4])
            nc.gpsimd.collective_compute(
                "AllReduce", AluOpType.add,
                replica_groups=[list(range(num_devices))],
                ins=[stats_in_d[:]], outs=[stats_out_d[:]])
            nc.sync.dma_start(SM[:, 8:12], stats_out_d[:])

            for oh in range(2):
                mean = SM[:, 16 + oh:17 + oh]
                ex2 = SM[:, 18 + oh:19 + oh]
                var = SM[:, 20 + oh:21 + oh]
                sd = SM[:, 22 + oh:23 + oh]
                rstd = SM[:, 24 + oh:25 + oh]
                s1 = SM[:, 26 + oh:27 + oh]
                ms = SM[:, 28 + oh:29 + oh]
                s2 = SM[:, 30 + oh:31 + oh]
                nc.vector.tensor_scalar(mean, SM[:, 8 + oh:9 + oh],
                                        1.0 / NTOT, None, AluOpType.mult)
                nc.vector.tensor_scalar(ex2, SM[:, 10 + oh:11 + oh],
                                        1.0 / NTOT, None, AluOpType.mult)
                nc.vector.tensor_tensor(var, mean, mean, AluOpType.mult)
                nc.vector.tensor_tensor(var, ex2, var, AluOpType.subtract)
                nc.scalar.activation(sd, var, AF.Sqrt, bias=EPSAP)
                nc.vector.reciprocal(rstd, sd)
                nc.vector.tensor_tensor(s1, GBt[:, oh, 0:1], rstd, AluOpType.mult)
                nc.vector.tensor_tensor(ms, mean, s1, AluOpType.mult)
                nc.vector.tensor_tensor(s2, GBt[:, oh, 1:2], ms, AluOpType.subtract)
                OUTS = otp.tile([128, HALF], F32, tag="o")
                nc.scalar.activation(OUTS[:], PSD[oh][:], AF.Relu,
                                     bias=s2, scale=s1)
                nc.sync.dma_start(out_d[128 * oh:128 * (oh + 1), :], OUTS[:])

    nc.compile()
    return nc


def host_inputs(x, w_off, b_off, w_def, gamma, beta):
    """Build the 8 per-core input dicts."""
    x = np.asarray(x, np.float32)
    w_off = np.asarray(w_off, np.float32)
    b_off = np.asarray(b_off, np.float32)
    w_def = np.asarray(w_def, np.float32)
    gamma = np.asarray(gamma, np.float32)
    beta = np.asarray(beta, np.float32)

    # weight stationaries, shared by all cores.
    woff = np.zeros((2, 128, K2, 18), np.float32)
    wdef = np.zeros((2, 128, K2, O), np.float32)
    for cg in range(2):
        cs = slice(128 * cg, 128 * (cg + 1))
        for kk in range(K2):
            ky, kx = kk // K, kk % K
            for j in range(K2):
                woff[cg, :, kk, j] = w_off[2 * j, cs, ky, kx]
                woff[cg, :, kk, 9 + j] = w_off[2 * j + 1, cs, ky, kx]
            wdef[cg, :, kk, :] = w_def[:, cs, ky, kx].T
    woff = np.asarray(woff.astype(ml_dtypes.bfloat16))
    wdef = np.asarray(wdef.astype(ml_dtypes.bfloat16))

    gb = np.zeros((2, 128, 2), np.float32)
    gb[0, :, 0], gb[1, :, 0] = gamma[:128], gamma[128:]
    gb[0, :, 1], gb[1, :, 1] = beta[:128], beta[128:]

    pl = np.arange(HALF, dtype=np.float32)
    hloc = np.floor(pl / W)
    wloc = pl % W
    kb = np.zeros((K2, 2, HALF), np.float32)
    for kk in range(K2):
        ky, kx = kk // K, kk % K
        kb[kk, 0, :] = 7.5 + (ky - 1) + hloc + b_off[2 * kk]
        kb[kk, 1, :] = 7.5 + (kx - 1) + wloc + b_off[2 * kk + 1]

    in_maps = []
    for core in range(NCORES):
        b, ph = core // 2, core % 2
        # gather table: 81x81 zero-extended channel-last grid; ext row r
        # corresponds to image row r - 8 + 32*ph, ext col c to image col c - 8
        E = np.zeros((81, 81, 256), np.float32)
        a_lo, a_hi = 32 * ph - 8, 32 * ph - 8 + 81
        sa_lo, sa_hi = max(0, a_lo), min(H, a_hi)
        E[sa_lo - a_lo:sa_hi - a_lo, 8:8 + W, :] = \
            x[b].transpose(1, 2, 0)[sa_lo:sa_hi, :, :]
        Eb = E.astype(ml_dtypes.bfloat16)
        blocks = []
        for cg in range(2):
            cs = slice(128 * cg, 128 * (cg + 1))
            for dy in (0, 1):
                for dx in (0, 1):
                    blocks.append(Eb[dy:dy + EG, dx:dx + EG, cs])
        gt = np.ascontiguousarray(
            np.concatenate(blocks, axis=2).reshape(NPOS, 1024))

        # conv input: rows 32*ph-1 .. 32*ph+32, cols -1..64, zero padded
        r_lo, r_hi = 32 * ph - 1, 32 * ph + 33
        s_lo, s_hi = max(0, r_lo), min(H, r_hi)
        xcv = np.zeros((C, 34, 66), np.float32)
        xcv[:, s_lo - r_lo:s_hi - r_lo, 1:65] = x[b][:, s_lo:s_hi, :]
        xc = np.ascontiguousarray(
            np.stack([xcv[:128], xcv[128:]]).astype(ml_dtypes.bfloat16))

        m = {"woff": woff, "wdef": wdef, "kb": kb, "gb": gb,
             "gt": gt, "xc": xc}
        in_maps.append(m)
    return in_maps


_prog_cache = {}


def _get_prog():
    if "nc" not in _prog_cache:
        _prog_cache["nc"] = build_program(NCORES)
    return _prog_cache["nc"]


def kernel(x, w_off, b_off, w_def, gamma, beta):
    nc = _get_prog()
    in_maps = host_inputs(x, w_off, b_off, w_def, gamma, beta)
    res = run_bass_kernel_spmd(nc, in_maps, core_ids=list(range(NCORES)))
    out = np.zeros((B, O, H, W), np.float32)
    for core in range(NCORES):
        b, ph = core // 2, core % 2
        out[b, :, 32 * ph:32 * (ph + 1), :] = \
            res.results[core]["out"].reshape(O, HROWS, W)
    return out
